# revision 15
# baseline (speedup 1.0000x reference)
"""Exact-IO bucketed Trainium2 kernel for sparse transposed conv + ReLU.

History: the original kernel was GPSIMD-gather-bound (ap_gather ~25ns/index
=> 2.5ms of a 2.95ms span). This design eliminates every on-device gather
and ships each parent feat row to the device exactly once (bf16).

Each output row j has exactly one (parent, offset k) contributor and each
parent has exactly CHILDREN=4 distinct offsets, so per parent the kernel
computes 4 GEMV-blocks: out[j(p,s)] = relu(feats[p] @ weight[k_s]).

  - Host buckets parents by their 4-offset subset (70 = C(8,4)), orders
    buckets along a Hamiltonian path of the Johnson graph J(8,4) (single
    swap between consecutive subsets), and assigns each bucket's 4 offsets
    to 4 PSUM "planes" so only the swapped offset changes plane at a
    transition. Matmuls merge into runs of constant (plane, k) spanning
    consecutive buckets. Parents are round-robined within each bucket
    across the 8 cores, so per-core bucket counts differ by <=1 and the
    uniform-quota padding is ~0.4% (the SPMD program is identical on all
    cores; quotas are the max per bucket).
  - Device: per 2048-parent tile, DMA x [128ci, 2, tile] bf16; per run a
    pair of 128-deep bf16 matmuls (ci halves, fp32 PSUM accumulate, one
    PSUM bank per run since start=True resets accumulation state at bank
    granularity); ReLU-copy PSUM -> bf16 y tile, alternating ScalarE and
    VectorE; DMA out [128co, 4, tile].
  - Host scatters y[co, plane, pos] to out rows (pure numpy fancy index).

Exactly N_OUT*C_IN*C_OUT MACs (no k-duplication), ~6.4MB in + ~12.9MB out
per core; TensorE-stream-bound at ~83us (36x over the 2.93ms baseline).
bf16 is safe: measured rel err 3.9e-3 vs the 2e-2 gate.
"""

import functools
import os

import numpy as np

N_IN = 100_000
K = 8
C_IN = 256
C_OUT = 128
CHILDREN = 4
N_OUT = N_IN * CHILDREN
NCORES = 8
RP = N_IN // NCORES       # parents per core (12500)
GROUP = 2048              # parents per DMA tile
PSRUN = 512               # max matmul cols (one PSUM bank)

LAST_RESULTS = None


def _gray_subsets():
    """Hamiltonian path over 4-subsets of {0..7}, consecutive differ by one
    swap (Johnson graph J(8,4)). Deterministic DFS."""
    from itertools import combinations
    subsets = [frozenset(c) for c in combinations(range(8), 4)]
    index = {s: i for i, s in enumerate(subsets)}
    nbr = [[] for _ in subsets]
    for i, s in enumerate(subsets):
        for j, t in enumerate(subsets):
            if i != j and len(s & t) == 3:
                nbr[i].append(j)
    n = len(subsets)
    path = [index[frozenset({0, 1, 2, 3})]]
    used = [False] * n
    used[path[0]] = True

    def dfs():
        if len(path) == n:
            return True
        # prefer low-degree-remaining neighbors (Warnsdorff) for fast success
        cands = [j for j in nbr[path[-1]] if not used[j]]
        cands.sort(key=lambda j: sum(not used[x] for x in nbr[j]))
        for j in cands:
            used[j] = True
            path.append(j)
            if dfs():
                return True
            path.pop()
            used[j] = False
        return False

    assert dfs(), "no Hamiltonian path found"
    ordered = [sorted(subsets[i]) for i in path]
    # plane assignment: start with sorted order; at each swap the removed
    # offset's plane takes the added offset
    planes = [list(ordered[0])]
    for prev, cur in zip(ordered, ordered[1:]):
        rem = (set(prev) - set(cur)).pop()
        add = (set(cur) - set(prev)).pop()
        p = list(planes[-1])
        p[p.index(rem)] = add
        planes.append(p)
    return ordered, planes


def _bounds(P: int):
    """DMA-tile boundaries: small head tiles so the first matmul is not
    gated on a 2MB transfer, then steady-state GROUP-sized tiles."""
    b = [0]
    for sz in (512, 512, 1024):
        if b[-1] + sz < P:
            b.append(b[-1] + sz)
    while b[-1] + GROUP < P:
        b.append(b[-1] + GROUP)
    b.append(P)
    return b


@functools.lru_cache(maxsize=1)
def _schedule(quotas: tuple, plane_ks: tuple):
    """Per DMA group, per plane: merged (a, b, k) runs chopped to <=PSRUN;
    columns are parent positions in the bucket-ordered layout."""
    P = sum(quotas)
    starts = np.cumsum((0,) + quotas)
    groups = []
    for g0, g1 in zip(_bounds(P), _bounds(P)[1:]):
        per_plane = []
        for p in range(4):
            runs = []
            for b, (s, q) in enumerate(zip(starts, quotas)):
                a, e = max(s, g0), min(s + q, g1)
                if a >= e:
                    continue
                k = plane_ks[b][p]
                if runs and runs[-1][2] == k and runs[-1][1] == a:
                    runs[-1] = (runs[-1][0], e, k)
                else:
                    runs.append((a, e, k))
            chopped = []
            for (a, e, k) in runs:
                for c0 in range(a, e, PSRUN):
                    chopped.append((c0, min(c0 + PSRUN, e), k))
            per_plane.append(chopped)
        groups.append(per_plane)
    return groups


@functools.lru_cache(maxsize=2)
def _build_program(quotas: tuple, plane_ks: tuple):
    from contextlib import ExitStack

    import concourse.tile as tile
    from concourse import bacc, mybir

    BF16 = mybir.dt.bfloat16
    F32 = mybir.dt.float32
    P = sum(quotas)
    groups = _schedule(quotas, plane_ks)

    nc = bacc.Bacc("TRN2", target_bir_lowering=False, debug=False,
                   num_devices=NCORES)
    xr_d = nc.dram_tensor("xr", [128, 2, P], BF16, kind="ExternalInput").ap()
    w2_d = nc.dram_tensor("w2", [128, K * 2 * C_OUT], BF16,
                          kind="ExternalInput").ap()
    y_d = nc.dram_tensor("y", [128, 4, P], BF16, kind="ExternalOutput").ap()

    with tile.TileContext(nc) as tc, ExitStack() as ctx:
        cpool = ctx.enter_context(tc.tile_pool(name="const", bufs=1))
        w2_s = cpool.tile([128, K * 2 * C_OUT], BF16)
        nc.sync.dma_start(out=w2_s[:], in_=w2_d[:])

        xpool = ctx.enter_context(tc.tile_pool(name="x", bufs=3))
        ypool = ctx.enter_context(tc.tile_pool(name="y", bufs=3))
        psmm = ctx.enter_context(tc.tile_pool(name="psmm", bufs=8,
                                              space="PSUM"))

        # PE warm-up: dummy matmuls on a zeroed tile run while the first x
        # DMA is in flight, so the HAM clock gate (needs ~3.4us of PE
        # activity) opens before the real matmul stream starts
        wpool = ctx.enter_context(tc.tile_pool(name="warm", bufs=1))
        xd = wpool.tile([128, PSRUN], BF16)
        scrap = wpool.tile([128, PSRUN], BF16)
        nc.vector.memset(xd[:], 0.0)
        for wi in range(8):
            # NB: assignee name sets the pool tag; must match the main
            # loop's `ps` so both share one 8-buf slot group
            ps = psmm.tile([128, PSRUN], F32)
            nc.tensor.matmul(out=ps[:], lhsT=xd[:, :128], rhs=xd[:],
                             start=True, stop=True)
            if wi % 2 == 0:
                nc.scalar.activation(
                    out=scrap[:], in_=ps[:],
                    func=mybir.ActivationFunctionType.Relu)
            else:
                nc.vector.tensor_scalar_max(scrap[:], ps[:], 0.0)

        bounds = _bounds(P)
        nrun = 0
        for gi, per_plane in enumerate(groups):
            g0 = bounds[gi]
            gsz = bounds[gi + 1] - g0
            xt = xpool.tile([128, 2, GROUP], BF16)
            nc.sync.dma_start(out=xt[:, :, :gsz], in_=xr_d[:, :, g0:g0 + gsz])
            yt = ypool.tile([128, 4, GROUP], BF16)
            for p, runs in enumerate(per_plane):
                # one PSUM tile per run: a start=True matmul resets
                # accumulation state at bank granularity, so each bank must
                # see exactly one start/stop pair before it is read
                for (a, e, k) in runs:
                    n = e - a
                    ps = psmm.tile([128, PSRUN], F32)
                    for h in (0, 1):
                        nc.tensor.matmul(
                            out=ps[:, :n],
                            lhsT=w2_s[:, (k * 2 + h) * C_OUT:
                                      (k * 2 + h + 1) * C_OUT],
                            rhs=xt[:, h, a - g0:e - g0],
                            start=(h == 0), stop=(h == 1))
                    dst = yt[:, p, a - g0:e - g0]
                    if nrun % 2 == 0:
                        nc.scalar.activation(
                            out=dst, in_=ps[:, :n],
                            func=mybir.ActivationFunctionType.Relu)
                    else:
                        nc.vector.tensor_scalar_max(dst, ps[:, :n], 0.0)
                    nrun += 1
            nc.sync.dma_start(out=y_d[:, :, g0:g0 + gsz],
                              in_=yt[:, :, :gsz])

    nc.compile()
    return nc


def _host_prep(feats, weight, gather_idx, scatter_idx, n_out):
    import ml_dtypes
    BF16 = ml_dtypes.bfloat16

    feats = np.asarray(feats, dtype=np.float32)
    weight = np.asarray(weight, dtype=np.float32)
    gather_idx = np.asarray(gather_idx, dtype=np.int64)
    scatter_idx = np.asarray(scatter_idx, dtype=np.int64)
    n_out = int(n_out)
    assert feats.shape == (N_IN, C_IN) and weight.shape == (K, C_IN, C_OUT)
    assert n_out == N_OUT

    feats_pad = np.concatenate([feats, np.zeros((1, C_IN), np.float32)],
                               axis=0)
    feats2b = np.ascontiguousarray(
        feats_pad.reshape(N_IN + 1, 2, 128).transpose(2, 1, 0)).astype(BF16)
    w2b = np.ascontiguousarray(
        weight.reshape(K, 2, 128, C_OUT).transpose(2, 0, 1, 3)
    ).reshape(128, K * 2 * C_OUT).astype(BF16)

    # flatten all real matches -> per-parent (4 sorted ks, their out rows)
    P_all, K_all, J_all = [], [], []
    for k in range(K):
        valid = scatter_idx[k] < n_out
        P_all.append(gather_idx[k][valid])
        J_all.append(scatter_idx[k][valid])
        K_all.append(np.full(valid.sum(), k, np.int64))
    P_all = np.concatenate(P_all)
    K_all = np.concatenate(K_all)
    J_all = np.concatenate(J_all)
    assert len(P_all) == N_OUT
    order = np.argsort(P_all, kind="stable")
    assert np.array_equal(P_all[order],
                          np.repeat(np.arange(N_IN), CHILDREN))
    Ks = K_all[order].reshape(N_IN, 4)
    Js = J_all[order].reshape(N_IN, 4)
    srt = np.argsort(Ks, axis=1)
    Ks = np.take_along_axis(Ks, srt, axis=1)          # sorted ks per parent
    Js = np.take_along_axis(Js, srt, axis=1)          # out rows, k-sorted
    assert (np.diff(Ks, axis=1) > 0).all(), "parent offsets not distinct"

    ordered_sets, planes = _gray_subsets()
    mask_to_b = {}
    for b, s in enumerate(ordered_sets):
        mask_to_b[sum(1 << k for k in s)] = b
    masks = (1 << Ks).sum(axis=1)
    b_of = np.vectorize(mask_to_b.__getitem__)(masks)  # bucket per parent

    # perm4[b][p] = rank of planes[b][p] within sorted set
    perm4 = np.empty((len(ordered_sets), 4), np.int64)
    for b, s in enumerate(ordered_sets):
        rank = {k: r for r, k in enumerate(s)}
        for p in range(4):
            perm4[b, p] = rank[planes[b][p]]

    # balanced sharding: round-robin parents within each bucket across
    # cores, so per-core bucket counts differ by <=1 and quota padding is
    # negligible (vs ~11% for contiguous parent ranges)
    nb = len(ordered_sets)
    o_global = np.argsort(b_of, kind="stable")       # parents bucket-major
    bb_g = b_of[o_global]
    bstart = np.searchsorted(bb_g, np.arange(nb), side="left")
    rank_g = np.arange(N_IN) - bstart[bb_g]          # rank within bucket
    total = np.bincount(b_of, minlength=nb)
    quotas = -(-total // NCORES)                     # ceil
    Ptot = int(quotas.sum())
    pad = (-Ptot) % 256
    quotas[-1] += pad
    Ptot += pad
    quotas = tuple(int(q) for q in quotas)
    starts = np.cumsum((0,) + quotas)

    core_g = rank_g % NCORES                         # core per bucket-rank
    pos_g = starts[bb_g] + rank_g // NCORES          # position in layout
    in_maps, unshard = [], []
    for c in range(NCORES):
        sel = core_g == c
        pars_o = o_global[sel]
        pos = pos_g[sel]
        bb_o = bb_g[sel]
        gidx = np.full(Ptot, N_IN, np.int64)
        gidx[pos] = pars_o
        in_maps.append({
            "xr": np.ascontiguousarray(feats2b[:, :, gidx]),
            "w2": w2b,
        })
        unshard.append((pos, pars_o, bb_o))
    plane_ks = tuple(tuple(p) for p in planes)
    return in_maps, quotas, plane_ks, unshard, Js, perm4


def _ensure_ntff_hook():
    import sys
    import types
    try:
        import antenv.axon_hooks  # noqa: F401
        return True
    except ImportError:
        pass
    try:
        import antenv
        from trn_agent_boot.trn_boot import _ntff_profile_via_ctypes
    except ImportError:
        return False
    mod = types.ModuleType("antenv.axon_hooks")
    holder = {}
    mod.set_axon_ntff_profile_hook = lambda h: holder.__setitem__("h", h)
    mod.get_axon_ntff_profile_hook = lambda: holder.get("h")
    sys.modules["antenv.axon_hooks"] = mod
    antenv.axon_hooks = mod
    try:
        h = _ntff_profile_via_ctypes("/opt/axon/libaxon_pjrt.so")
    except OSError:
        h = None
    if h is not None:
        mod.set_axon_ntff_profile_hook(h)
    return True


def _simulate(in_maps, quotas, plane_ks, w2b):
    """Numpy mirror of the device program (same schedule), for validation."""
    groups = _schedule(quotas, plane_ks)
    w = w2b.astype(np.float32)
    ys = []
    for m in in_maps:
        x = m["xr"].astype(np.float32)          # [128, 2, P]
        P = x.shape[2]
        y = np.zeros((128, 4, P), np.float32)
        for gi, per_plane in enumerate(groups):
            for p, runs in enumerate(per_plane):
                for (a, e, k) in runs:
                    acc = (w[:, (k * 2 + 0) * C_OUT:(k * 2 + 1) * C_OUT].T
                           @ x[:, 0, a:e])
                    acc += (w[:, (k * 2 + 1) * C_OUT:(k * 2 + 2) * C_OUT].T
                            @ x[:, 1, a:e])
                    y[:, p, a:e] = np.maximum(acc, 0.0)
        import ml_dtypes
        ys.append(y.astype(ml_dtypes.bfloat16))
    return ys


def kernel(**inputs):
    global LAST_RESULTS
    in_maps, quotas, plane_ks, unshard, Js, perm4 = _host_prep(
        inputs["feats"], inputs["weight"], inputs["gather_idx"],
        inputs["scatter_idx"], inputs["n_out"])

    if os.environ.get("KERNEL_SIMULATE", "0") == "1":
        ys = _simulate(in_maps, quotas, plane_ks, in_maps[0]["w2"])
        results = [{"y": y} for y in ys]
    else:
        from concourse.bass_utils import run_bass_kernel_spmd
        nc = _build_program(quotas, plane_ks)
        trace = bool(int(os.environ.get("KERNEL_TRACE", "0")))
        if trace:
            trace = _ensure_ntff_hook()
        res = run_bass_kernel_spmd(nc, in_maps, list(range(NCORES)),
                                   trace=trace)
        LAST_RESULTS = res
        results = res.results

    out = np.zeros((N_OUT, C_OUT), np.float32)
    for c in range(NCORES):
        y = np.asarray(results[c]["y"])          # [128, 4, P] bf16
        pos, pars_o, bb_o = unshard[c]
        for p in range(4):
            rows = Js[pars_o, perm4[bb_o, p]]
            out[rows] = y[:, p, pos].T.astype(np.float32)
    return out


# revision 16
# speedup vs baseline: 1.1101x; 1.1101x over previous
"""Exact-IO bucketed Trainium2 kernel for sparse transposed conv + ReLU.

History: the original kernel was GPSIMD-gather-bound (ap_gather ~25ns/index
=> 2.5ms of a 2.95ms span). This design eliminates every on-device gather
and ships each parent feat row to the device exactly once (bf16).

Each output row j has exactly one (parent, offset k) contributor and each
parent has exactly CHILDREN=4 distinct offsets, so per parent the kernel
computes 4 GEMV-blocks: out[j(p,s)] = relu(feats[p] @ weight[k_s]).

  - Host buckets parents by their 4-offset subset (70 = C(8,4)), orders
    buckets along a Hamiltonian path of the Johnson graph J(8,4) (single
    swap between consecutive subsets), and assigns each bucket's 4 offsets
    to 4 PSUM "planes" so only the swapped offset changes plane at a
    transition. Matmuls merge into runs of constant (plane, k) spanning
    consecutive buckets. Parents are round-robined within each bucket
    across the 8 cores, so per-core bucket counts differ by <=1 and the
    uniform-quota padding is ~0.4% (the SPMD program is identical on all
    cores; quotas are the max per bucket).
  - Device: per 2048-parent tile, DMA x [128ci, 2, tile] bf16; per run a
    pair of 128-deep bf16 matmuls (ci halves, fp32 PSUM accumulate, one
    PSUM bank per run since start=True resets accumulation state at bank
    granularity); ReLU-copy PSUM -> bf16 y tile, alternating ScalarE and
    VectorE; DMA out [128co, 4, tile].
  - Host scatters y[co, plane, pos] to out rows (pure numpy fancy index).

Exactly N_OUT*C_IN*C_OUT MACs (no k-duplication), ~6.4MB in + ~12.9MB out
per core; TensorE-stream-bound at ~83us (36x over the 2.93ms baseline).
bf16 is safe: measured rel err 3.9e-3 vs the 2e-2 gate.
"""

import functools
import os

import numpy as np

N_IN = 100_000
K = 8
C_IN = 256
C_OUT = 128
CHILDREN = 4
N_OUT = N_IN * CHILDREN
NCORES = 8
RP = N_IN // NCORES       # parents per core (12500)
GROUP = 2048              # parents per DMA tile
PSRUN = 512               # max matmul cols (one PSUM bank)

LAST_RESULTS = None


def _gray_subsets():
    """Hamiltonian path over 4-subsets of {0..7}, consecutive differ by one
    swap (Johnson graph J(8,4)). Deterministic DFS."""
    from itertools import combinations
    subsets = [frozenset(c) for c in combinations(range(8), 4)]
    index = {s: i for i, s in enumerate(subsets)}
    nbr = [[] for _ in subsets]
    for i, s in enumerate(subsets):
        for j, t in enumerate(subsets):
            if i != j and len(s & t) == 3:
                nbr[i].append(j)
    n = len(subsets)
    path = [index[frozenset({0, 1, 2, 3})]]
    used = [False] * n
    used[path[0]] = True

    def dfs():
        if len(path) == n:
            return True
        # prefer low-degree-remaining neighbors (Warnsdorff) for fast success
        cands = [j for j in nbr[path[-1]] if not used[j]]
        cands.sort(key=lambda j: sum(not used[x] for x in nbr[j]))
        for j in cands:
            used[j] = True
            path.append(j)
            if dfs():
                return True
            path.pop()
            used[j] = False
        return False

    assert dfs(), "no Hamiltonian path found"
    ordered = [sorted(subsets[i]) for i in path]
    # plane assignment: start with sorted order; at each swap the removed
    # offset's plane takes the added offset
    planes = [list(ordered[0])]
    for prev, cur in zip(ordered, ordered[1:]):
        rem = (set(prev) - set(cur)).pop()
        add = (set(cur) - set(prev)).pop()
        p = list(planes[-1])
        p[p.index(rem)] = add
        planes.append(p)
    return ordered, planes


def _bounds(P: int):
    """DMA-tile boundaries: small head tiles so the first matmul is not
    gated on a 2MB transfer, then steady-state GROUP-sized tiles."""
    b = [0]
    for sz in (512, 512, 1024):
        if b[-1] + sz < P:
            b.append(b[-1] + sz)
    while b[-1] + GROUP < P:
        b.append(b[-1] + GROUP)
    b.append(P)
    return b


@functools.lru_cache(maxsize=1)
def _schedule(quotas: tuple, plane_ks: tuple):
    """Per DMA group, per plane: merged (a, b, k) runs chopped to <=PSRUN;
    columns are parent positions in the bucket-ordered layout."""
    P = sum(quotas)
    starts = np.cumsum((0,) + quotas)
    groups = []
    for g0, g1 in zip(_bounds(P), _bounds(P)[1:]):
        per_plane = []
        for p in range(4):
            runs = []
            for b, (s, q) in enumerate(zip(starts, quotas)):
                a, e = max(s, g0), min(s + q, g1)
                if a >= e:
                    continue
                k = plane_ks[b][p]
                if runs and runs[-1][2] == k and runs[-1][1] == a:
                    runs[-1] = (runs[-1][0], e, k)
                else:
                    runs.append((a, e, k))
            chopped = []
            for (a, e, k) in runs:
                for c0 in range(a, e, PSRUN):
                    chopped.append((c0, min(c0 + PSRUN, e), k))
            per_plane.append(chopped)
        groups.append(per_plane)
    return groups


@functools.lru_cache(maxsize=2)
def _build_program(quotas: tuple, plane_ks: tuple):
    from contextlib import ExitStack

    import concourse.tile as tile
    from concourse import bacc, mybir

    BF16 = mybir.dt.bfloat16
    F32 = mybir.dt.float32
    P = sum(quotas)
    groups = _schedule(quotas, plane_ks)

    nc = bacc.Bacc("TRN2", target_bir_lowering=False, debug=False,
                   num_devices=NCORES)
    xr_d = nc.dram_tensor("xr", [128, 2, P], BF16, kind="ExternalInput").ap()
    w2_d = nc.dram_tensor("w2", [128, K * 2 * C_OUT], BF16,
                          kind="ExternalInput").ap()
    y_d = nc.dram_tensor("y", [128, 4, P], BF16, kind="ExternalOutput").ap()

    with tile.TileContext(nc) as tc, ExitStack() as ctx:
        cpool = ctx.enter_context(tc.tile_pool(name="const", bufs=1))
        w2_s = cpool.tile([128, K * 2 * C_OUT], BF16)
        nc.sync.dma_start(out=w2_s[:], in_=w2_d[:])

        xpool = ctx.enter_context(tc.tile_pool(name="x", bufs=3))
        ypool = ctx.enter_context(tc.tile_pool(name="y", bufs=3))
        psmm = ctx.enter_context(tc.tile_pool(name="psmm", bufs=8,
                                              space="PSUM"))

        bounds = _bounds(P)
        nrun = 0
        for gi, per_plane in enumerate(groups):
            g0 = bounds[gi]
            gsz = bounds[gi + 1] - g0
            xt = xpool.tile([128, 2, GROUP], BF16)
            nc.sync.dma_start(out=xt[:, :, :gsz], in_=xr_d[:, :, g0:g0 + gsz])
            yt = ypool.tile([128, 4, GROUP], BF16)
            for p, runs in enumerate(per_plane):
                # one PSUM tile per run: a start=True matmul resets
                # accumulation state at bank granularity, so each bank must
                # see exactly one start/stop pair before it is read
                for (a, e, k) in runs:
                    n = e - a
                    ps = psmm.tile([128, PSRUN], F32)
                    for h in (0, 1):
                        nc.tensor.matmul(
                            out=ps[:, :n],
                            lhsT=w2_s[:, (k * 2 + h) * C_OUT:
                                      (k * 2 + h + 1) * C_OUT],
                            rhs=xt[:, h, a - g0:e - g0],
                            start=(h == 0), stop=(h == 1))
                    # split evacuation across both engines: halves PSUM
                    # hold latency, so the matmul stream stalls less on
                    # bank rotation
                    m = n // 2
                    if m >= 64:
                        if nrun % 2 == 0:
                            nc.scalar.activation(
                                out=yt[:, p, a - g0:a - g0 + m],
                                in_=ps[:, :m],
                                func=mybir.ActivationFunctionType.Relu)
                            nc.vector.tensor_scalar_max(
                                yt[:, p, a - g0 + m:e - g0], ps[:, m:n], 0.0)
                        else:
                            nc.vector.tensor_scalar_max(
                                yt[:, p, a - g0:a - g0 + m], ps[:, :m], 0.0)
                            nc.scalar.activation(
                                out=yt[:, p, a - g0 + m:e - g0],
                                in_=ps[:, m:n],
                                func=mybir.ActivationFunctionType.Relu)
                    elif nrun % 2 == 0:
                        nc.scalar.activation(
                            out=yt[:, p, a - g0:e - g0], in_=ps[:, :n],
                            func=mybir.ActivationFunctionType.Relu)
                    else:
                        nc.vector.tensor_scalar_max(
                            yt[:, p, a - g0:e - g0], ps[:, :n], 0.0)
                    nrun += 1
            nc.sync.dma_start(out=y_d[:, :, g0:g0 + gsz],
                              in_=yt[:, :, :gsz])

    nc.compile()
    return nc


def _host_prep(feats, weight, gather_idx, scatter_idx, n_out):
    import ml_dtypes
    BF16 = ml_dtypes.bfloat16

    feats = np.asarray(feats, dtype=np.float32)
    weight = np.asarray(weight, dtype=np.float32)
    gather_idx = np.asarray(gather_idx, dtype=np.int64)
    scatter_idx = np.asarray(scatter_idx, dtype=np.int64)
    n_out = int(n_out)
    assert feats.shape == (N_IN, C_IN) and weight.shape == (K, C_IN, C_OUT)
    assert n_out == N_OUT

    feats_pad = np.concatenate([feats, np.zeros((1, C_IN), np.float32)],
                               axis=0)
    feats2b = np.ascontiguousarray(
        feats_pad.reshape(N_IN + 1, 2, 128).transpose(2, 1, 0)).astype(BF16)
    w2b = np.ascontiguousarray(
        weight.reshape(K, 2, 128, C_OUT).transpose(2, 0, 1, 3)
    ).reshape(128, K * 2 * C_OUT).astype(BF16)

    # flatten all real matches -> per-parent (4 sorted ks, their out rows)
    P_all, K_all, J_all = [], [], []
    for k in range(K):
        valid = scatter_idx[k] < n_out
        P_all.append(gather_idx[k][valid])
        J_all.append(scatter_idx[k][valid])
        K_all.append(np.full(valid.sum(), k, np.int64))
    P_all = np.concatenate(P_all)
    K_all = np.concatenate(K_all)
    J_all = np.concatenate(J_all)
    assert len(P_all) == N_OUT
    order = np.argsort(P_all, kind="stable")
    assert np.array_equal(P_all[order],
                          np.repeat(np.arange(N_IN), CHILDREN))
    Ks = K_all[order].reshape(N_IN, 4)
    Js = J_all[order].reshape(N_IN, 4)
    srt = np.argsort(Ks, axis=1)
    Ks = np.take_along_axis(Ks, srt, axis=1)          # sorted ks per parent
    Js = np.take_along_axis(Js, srt, axis=1)          # out rows, k-sorted
    assert (np.diff(Ks, axis=1) > 0).all(), "parent offsets not distinct"

    ordered_sets, planes = _gray_subsets()
    mask_to_b = {}
    for b, s in enumerate(ordered_sets):
        mask_to_b[sum(1 << k for k in s)] = b
    masks = (1 << Ks).sum(axis=1)
    b_of = np.vectorize(mask_to_b.__getitem__)(masks)  # bucket per parent

    # perm4[b][p] = rank of planes[b][p] within sorted set
    perm4 = np.empty((len(ordered_sets), 4), np.int64)
    for b, s in enumerate(ordered_sets):
        rank = {k: r for r, k in enumerate(s)}
        for p in range(4):
            perm4[b, p] = rank[planes[b][p]]

    # balanced sharding: round-robin parents within each bucket across
    # cores, so per-core bucket counts differ by <=1 and quota padding is
    # negligible (vs ~11% for contiguous parent ranges)
    nb = len(ordered_sets)
    o_global = np.argsort(b_of, kind="stable")       # parents bucket-major
    bb_g = b_of[o_global]
    bstart = np.searchsorted(bb_g, np.arange(nb), side="left")
    rank_g = np.arange(N_IN) - bstart[bb_g]          # rank within bucket
    total = np.bincount(b_of, minlength=nb)
    quotas = -(-total // NCORES)                     # ceil
    Ptot = int(quotas.sum())
    pad = (-Ptot) % 256
    quotas[-1] += pad
    Ptot += pad
    quotas = tuple(int(q) for q in quotas)
    starts = np.cumsum((0,) + quotas)

    core_g = rank_g % NCORES                         # core per bucket-rank
    pos_g = starts[bb_g] + rank_g // NCORES          # position in layout
    in_maps, unshard = [], []
    for c in range(NCORES):
        sel = core_g == c
        pars_o = o_global[sel]
        pos = pos_g[sel]
        bb_o = bb_g[sel]
        gidx = np.full(Ptot, N_IN, np.int64)
        gidx[pos] = pars_o
        in_maps.append({
            "xr": np.ascontiguousarray(feats2b[:, :, gidx]),
            "w2": w2b,
        })
        unshard.append((pos, pars_o, bb_o))
    plane_ks = tuple(tuple(p) for p in planes)
    return in_maps, quotas, plane_ks, unshard, Js, perm4


def _ensure_ntff_hook():
    import sys
    import types
    try:
        import antenv.axon_hooks  # noqa: F401
        return True
    except ImportError:
        pass
    try:
        import antenv
        from trn_agent_boot.trn_boot import _ntff_profile_via_ctypes
    except ImportError:
        return False
    mod = types.ModuleType("antenv.axon_hooks")
    holder = {}
    mod.set_axon_ntff_profile_hook = lambda h: holder.__setitem__("h", h)
    mod.get_axon_ntff_profile_hook = lambda: holder.get("h")
    sys.modules["antenv.axon_hooks"] = mod
    antenv.axon_hooks = mod
    try:
        h = _ntff_profile_via_ctypes("/opt/axon/libaxon_pjrt.so")
    except OSError:
        h = None
    if h is not None:
        mod.set_axon_ntff_profile_hook(h)
    return True


def _simulate(in_maps, quotas, plane_ks, w2b):
    """Numpy mirror of the device program (same schedule), for validation."""
    groups = _schedule(quotas, plane_ks)
    w = w2b.astype(np.float32)
    ys = []
    for m in in_maps:
        x = m["xr"].astype(np.float32)          # [128, 2, P]
        P = x.shape[2]
        y = np.zeros((128, 4, P), np.float32)
        for gi, per_plane in enumerate(groups):
            for p, runs in enumerate(per_plane):
                for (a, e, k) in runs:
                    acc = (w[:, (k * 2 + 0) * C_OUT:(k * 2 + 1) * C_OUT].T
                           @ x[:, 0, a:e])
                    acc += (w[:, (k * 2 + 1) * C_OUT:(k * 2 + 2) * C_OUT].T
                            @ x[:, 1, a:e])
                    y[:, p, a:e] = np.maximum(acc, 0.0)
        import ml_dtypes
        ys.append(y.astype(ml_dtypes.bfloat16))
    return ys


def kernel(**inputs):
    global LAST_RESULTS
    in_maps, quotas, plane_ks, unshard, Js, perm4 = _host_prep(
        inputs["feats"], inputs["weight"], inputs["gather_idx"],
        inputs["scatter_idx"], inputs["n_out"])

    if os.environ.get("KERNEL_SIMULATE", "0") == "1":
        ys = _simulate(in_maps, quotas, plane_ks, in_maps[0]["w2"])
        results = [{"y": y} for y in ys]
    else:
        from concourse.bass_utils import run_bass_kernel_spmd
        nc = _build_program(quotas, plane_ks)
        trace = bool(int(os.environ.get("KERNEL_TRACE", "0")))
        if trace:
            trace = _ensure_ntff_hook()
        res = run_bass_kernel_spmd(nc, in_maps, list(range(NCORES)),
                                   trace=trace)
        LAST_RESULTS = res
        results = res.results

    out = np.zeros((N_OUT, C_OUT), np.float32)
    for c in range(NCORES):
        y = np.asarray(results[c]["y"])          # [128, 4, P] bf16
        pos, pars_o, bb_o = unshard[c]
        for p in range(4):
            rows = Js[pars_o, perm4[bb_o, p]]
            out[rows] = y[:, p, pos].T.astype(np.float32)
    return out
